# revision 5
# baseline (speedup 1.0000x reference)
"""DLRM forward (embedding gather + tiny MLPs) as a Bass/Tile kernel on 8 trn2 cores.

Sharding: data-parallel over the batch (tables replicated in each core's HBM —
total gather traffic is the same as model-parallel but needs no collectives).
Each core computes 2048 rows end-to-end and returns [1, 2048] sigmoid outputs.

v2 (v1 was 68.7µs, v0 110µs):
  - all host-prep of v1 (combined gather indices, bias-augmented fp16
    weights) plus host pre-transposed layouts for the index / dense / tw1
    DMAs so every weight DMA is a contiguous 128-partition transfer
    (v1's scattered index DMA alone cost ~3µs before the first gather).
  - the feature transposes moved off the PE onto the DMA XBAR
    (dma_start_transpose, issued from the sync+scalar HWDGE queues):
    each [128,1664] gather tile is transposed SBUF->SBUF into a
    [128(k), 13(chunk), 4(tile), 128(sample)] buffer in one instruction.
    This removes 208 PE transposes AND all 52 PSUM->SBUF casts; the PE
    stream is pure 512-column matmul accumulation and the top-MLP rhs
    for chunk kc is the contiguous block embT[:, kc, :, :].
  - bottom MLP (fp16, biases folded via ones-rows) runs up front under
    the first gather; ACT does only sigmoid; DVE only relu/small copies.
"""

import numpy as np

import concourse.bass as bass
import concourse.mybir as mybir
import concourse.tile as tile
from concourse import bacc

P = 128

N_CORES = 8
B = 16384
F = 26
D = 64
DENSE = 13
CARD = 100000
H_BOT = 8
H_TOP = 16

f32 = mybir.dt.float32
i32 = mybir.dt.int32
fp16 = mybir.dt.float16

B_LOC = B // N_CORES          # 2048
K_EMB = F * D                 # 1664
KC_N = K_EMB // P             # 13
GROUP = 512
TPG = GROUP // P              # 4
N_G = B_LOC // GROUP          # 4
N_T = B_LOC // P              # 16


def build_kernel():
    nc = bacc.Bacc("TRN2", target_bir_lowering=False)
    comb_d = nc.dram_tensor("sparse_x", [P, N_T * F], i32, kind="ExternalInput")
    dense_d = nc.dram_tensor("dense_x", [P, N_T * DENSE], fp16, kind="ExternalInput")
    tables_d = nc.dram_tensor("tables", [F * CARD, D], fp16, kind="ExternalInput")
    ident_d = nc.dram_tensor("ident", [P, P], fp16, kind="ExternalInput")
    w1a_d = nc.dram_tensor("w1a", [DENSE + 1, H_BOT], fp16, kind="ExternalInput")
    w2a_d = nc.dram_tensor("w2a", [H_BOT + 1, D], fp16, kind="ExternalInput")
    tw1c_d = nc.dram_tensor("tw1c", [P, KC_N * H_TOP], fp16, kind="ExternalInput")
    tw1da_d = nc.dram_tensor("tw1da", [D + 1, H_TOP], fp16, kind="ExternalInput")
    tw2_d = nc.dram_tensor("tw2", [H_TOP, 1], fp16, kind="ExternalInput")
    tb2_d = nc.dram_tensor("tb2", [1], f32, kind="ExternalInput")
    y_d = nc.dram_tensor("y", [1, B_LOC], f32, kind="ExternalOutput")

    with tile.TileContext(nc) as tc:
        with (
            tc.tile_pool(name="const", bufs=1) as cpool,
            tc.tile_pool(name="embT", bufs=2) as embtp,
            tc.tile_pool(name="small", bufs=3) as smallp,
            tc.tile_pool(name="po1", bufs=2, space="PSUM") as po1p,
            tc.tile_pool(name="psmall", bufs=3, space="PSUM") as psmallp,
        ):
            # ---- index DMA first (contiguous, host-pre-transposed) ----
            comb_sb = cpool.tile([P, N_T * F], i32)
            nc.sync.dma_start(out=comb_sb[:], in_=comb_d[:, :])

            # ---- gather dispatches (gpsimd only): group 0 split for latency ----
            embs = [cpool.tile([P, TPG * K_EMB], fp16, name=f"emb{g}") for g in range(N_G)]
            spans = [(0, 0, 2 * F), (0, 2 * F, 4 * F)] + [
                (g, 0, TPG * F) for g in range(1, N_G)
            ]
            for g, f0, f1 in spans:
                nc.gpsimd.indirect_dma_start(
                    out=embs[g][:, f0 * D : f1 * D],
                    out_offset=None,
                    in_=tables_d[:, :],
                    in_offset=bass.IndirectOffsetOnAxis(
                        ap=comb_sb[:, g * TPG * F + f0 : g * TPG * F + f1], axis=0
                    ),
                )

            # ---- weights / constants (host-prepped fp16, all contiguous) ----
            dense_sb = cpool.tile([P, N_T * DENSE], fp16)
            nc.sync.dma_start(out=dense_sb[:], in_=dense_d[:, :])
            ident = cpool.tile([P, P], fp16)
            nc.sync.dma_start(out=ident[:], in_=ident_d[:, :])
            tw1_c = cpool.tile([P, KC_N * H_TOP], fp16)
            nc.sync.dma_start(out=tw1_c[:], in_=tw1c_d[:, :])
            tw1da_sb = cpool.tile([D + 1, H_TOP], fp16)
            nc.sync.dma_start(out=tw1da_sb[:], in_=tw1da_d[:, :])
            w1a_sb = cpool.tile([DENSE + 1, H_BOT], fp16)
            nc.sync.dma_start(out=w1a_sb[:], in_=w1a_d[:, :])
            w2a_sb = cpool.tile([H_BOT + 1, D], fp16)
            nc.sync.dma_start(out=w2a_sb[:], in_=w2a_d[:, :])
            tw2_sb = cpool.tile([H_TOP, 1], fp16)
            nc.sync.dma_start(out=tw2_sb[:], in_=tw2_d[:, :])
            tb2_sb = cpool.tile([1, 1], f32)
            nc.sync.dma_start(out=tb2_sb[:], in_=tb2_d[:, None])

            y_row = cpool.tile([1, B_LOC], f32)
            dta = [cpool.tile([D + 1, GROUP], fp16, name=f"dta{g}") for g in range(N_G)]

            # ---- bottom MLP for all groups up front (PE idles during the
            # first gather anyway); biases folded in via ones-rows ----
            def dense_tr(g):
                pdx_t = psmallp.tile([DENSE, GROUP], fp16, tag="psmall")
                for j in range(TPG):
                    t = g * TPG + j
                    nc.tensor.transpose(
                        out=pdx_t[:, bass.ts(j, P)],
                        in_=dense_sb[:, bass.ts(t, DENSE)],
                        identity=ident[:],
                    )
                return pdx_t

            def dense_mm1(g, pdx_t):
                dxt = smallp.tile([DENSE + 1, GROUP], fp16, tag="dxt")
                nc.vector.memset(dxt[:], 1.0)
                nc.vector.tensor_copy(out=dxt[0:DENSE, :], in_=pdx_t[:])
                ph = psmallp.tile([H_BOT, GROUP], f32, tag="psmall")
                nc.tensor.matmul(out=ph[:], lhsT=w1a_sb[:], rhs=dxt[:], start=True, stop=True)
                return ph

            def dense_mm2(g, ph):
                ha = smallp.tile([H_BOT + 1, GROUP], fp16, tag="ha")
                nc.vector.memset(ha[:], 1.0)
                nc.vector.tensor_scalar_max(ha[0:H_BOT, :], ph[:], 0.0)
                pd = psmallp.tile([D, GROUP], f32, tag="psmall")
                nc.tensor.matmul(out=pd[:], lhsT=w2a_sb[:], rhs=ha[:], start=True, stop=True)
                nc.vector.memset(dta[g][:], 1.0)
                nc.vector.tensor_copy(out=dta[g][0:D, :], in_=pd[:])

            pdx0 = dense_tr(0)
            pdx1 = dense_tr(1)
            ph0 = dense_mm1(0, pdx0)
            pdx2 = dense_tr(2)
            ph1 = dense_mm1(1, pdx1)
            dense_mm2(0, ph0)
            pdx3 = dense_tr(3)
            ph2 = dense_mm1(2, pdx2)
            dense_mm2(1, ph1)
            ph3 = dense_mm1(3, pdx3)
            dense_mm2(2, ph2)
            dense_mm2(3, ph3)

            # ---- XBAR transposes + top-MLP accumulation ----
            tail = []  # deferred (o1 relu, tw2 matmul, sigmoid) of prev group

            def flush_tail():
                while tail:
                    tail.pop(0)()

            for g in range(N_G):
                # one XBAR transpose per 128-sample tile:
                #   embt[p, c, j, s] = embs[g][s, j*1664 + c*128 + p]
                embt = embtp.tile([P, KC_N, TPG, P], fp16, tag="embT")
                for j in range(TPG):
                    eng = nc.sync if j % 2 == 0 else nc.scalar
                    eng.dma_start_transpose(
                        out=embt[:, :, j, :],
                        in_=embs[g][:, j * K_EMB : (j + 1) * K_EMB],
                    )

                po1 = po1p.tile([H_TOP, GROUP], f32, tag="po1")
                for kc in range(KC_N):
                    nc.tensor.matmul(
                        out=po1[:],
                        lhsT=tw1_c[:, bass.ts(kc, H_TOP)],
                        rhs=embt[:, kc, :, :],
                        start=(kc == 0),
                        stop=False,
                    )
                    if kc == 1:
                        flush_tail()  # prev group's tw2 matmul fills the gap
                nc.tensor.matmul(
                    out=po1[:], lhsT=tw1da_sb[:], rhs=dta[g][:], start=False, stop=True
                )

                def make_tail(g=g, po1=po1):
                    def run():
                        o1 = smallp.tile([H_TOP, GROUP], fp16, tag="o1")
                        nc.vector.tensor_scalar_max(o1[:], po1[:], 0.0)
                        plg = psmallp.tile([1, GROUP], f32, tag="psmall")
                        nc.tensor.matmul(
                            out=plg[:], lhsT=tw2_sb[:], rhs=o1[:], start=True, stop=True
                        )
                        nc.scalar.activation(
                            out=y_row[:, bass.ts(g, GROUP)],
                            in_=plg[:],
                            func=mybir.ActivationFunctionType.Sigmoid,
                            bias=tb2_sb[:],
                        )
                    return run

                tail.append(make_tail())
            flush_tail()

            nc.sync.dma_start(out=y_d[:, :], in_=y_row[:])

    nc.compile()
    return nc


_NC_CACHE = {}


def _get_nc():
    if "nc" not in _NC_CACHE:
        _NC_CACHE["nc"] = build_kernel()
    return _NC_CACHE["nc"]


def make_in_maps(dense_x, sparse_x, tables, w1, b1, w2, b2, tw1, tb1, tw2, tb2):
    tables_flat = np.ascontiguousarray(
        np.asarray(tables).reshape(F * CARD, D).astype(np.float16)
    )
    comb = np.asarray(sparse_x).astype(np.int32) + (
        np.arange(F, dtype=np.int32) * CARD
    )[None, :]
    dense_f = np.asarray(dense_x).astype(np.float16)
    tw1 = np.asarray(tw1, np.float32)
    # tw1c[p, (c m)] = tw1[c*128+p, m]
    tw1c = np.ascontiguousarray(
        tw1[:K_EMB].astype(np.float16).reshape(KC_N, P, H_TOP).transpose(1, 0, 2).reshape(P, KC_N * H_TOP)
    )
    shared = {
        "tables": tables_flat,
        "ident": np.eye(P, dtype=np.float16),
        "w1a": np.vstack([np.asarray(w1, np.float32), np.asarray(b1, np.float32)[None, :]]).astype(np.float16),
        "w2a": np.vstack([np.asarray(w2, np.float32), np.asarray(b2, np.float32)[None, :]]).astype(np.float16),
        "tw1c": tw1c,
        "tw1da": np.vstack([tw1[K_EMB : K_EMB + D], np.asarray(tb1, np.float32)[None, :]]).astype(np.float16),
        "tw2": np.asarray(tw2, np.float32).astype(np.float16),
        "tb2": np.ascontiguousarray(np.asarray(tb2, np.float32)),
    }
    in_maps = []
    for c in range(N_CORES):
        m = dict(shared)
        # host pre-transpose: [p, (t f)] so the device DMA is contiguous
        dl = dense_f[c * B_LOC : (c + 1) * B_LOC]
        m["dense_x"] = np.ascontiguousarray(
            dl.reshape(N_T, P, DENSE).transpose(1, 0, 2).reshape(P, N_T * DENSE)
        )
        cl = comb[c * B_LOC : (c + 1) * B_LOC]
        m["sparse_x"] = np.ascontiguousarray(
            cl.reshape(N_T, P, F).transpose(1, 0, 2).reshape(P, N_T * F)
        )
        in_maps.append(m)
    return in_maps


def kernel(**inputs):
    from concourse.bass_utils import run_bass_kernel_spmd

    nc = _get_nc()
    in_maps = make_in_maps(**inputs)
    res = run_bass_kernel_spmd(nc, in_maps, core_ids=list(range(N_CORES)))
    out = np.concatenate([r["y"].reshape(-1) for r in res.results])
    return out.reshape(B, 1).astype(np.float32)


# revision 7
# speedup vs baseline: 1.2591x; 1.2591x over previous
"""DLRM forward (embedding gather + tiny MLPs) as a Bass/Tile kernel on 8 trn2 cores.

Sharding: data-parallel over the batch (tables replicated in each core's HBM —
total gather traffic is the same as model-parallel but needs no collectives).
Each core computes 2048 rows end-to-end and returns [1, 2048] sigmoid outputs.

v3 (v1 68.7µs, v2 78.7µs — XBAR dma-transpose was a loss, v0 110µs):
  - host prep: combined gather indices (idx + f*CARD) pre-arranged to the
    on-chip [128, (tile idx)] layout, dense/tw1 likewise, bias-augmented
    fp16 weights, identity matrix — every device DMA is contiguous and the
    device does zero index arithmetic or weight staging.
  - indirect gathers dispatched per 512-sample group (group 0 split in two
    for startup latency), all group buffers resident in SBUF.
  - PE does per-128-chunk transposes (fp16, PSUM) which DVE/ACT cast to
    SBUF; top-MLP accumulates 512-column fp16 matmuls; chunks are
    processed in PAIRS (8 transposes then 2 matmuls) to halve the
    PE matmul<->transpose mode-switch penalty (~106ns each).
  - bottom MLP (fp16, biases folded via ones-rows) runs up front under
    the first gather; group tails (relu+tw2+sigmoid) deferred into the
    next group's stream so the PE never waits on DVE/ACT.
"""

import numpy as np

import concourse.bass as bass
import concourse.mybir as mybir
import concourse.tile as tile
from concourse import bacc

P = 128

N_CORES = 8
B = 16384
F = 26
D = 64
DENSE = 13
CARD = 100000
H_BOT = 8
H_TOP = 16

f32 = mybir.dt.float32
i32 = mybir.dt.int32
fp16 = mybir.dt.float16

B_LOC = B // N_CORES          # 2048
K_EMB = F * D                 # 1664
KC_N = K_EMB // P             # 13
GROUP = 512
TPG = GROUP // P              # 4
N_G = B_LOC // GROUP          # 4
N_T = B_LOC // P              # 16


def build_kernel():
    nc = bacc.Bacc("TRN2", target_bir_lowering=False)
    comb_d = nc.dram_tensor("sparse_x", [P, N_T * F], i32, kind="ExternalInput")
    dense_d = nc.dram_tensor("dense_x", [P, N_T * DENSE], fp16, kind="ExternalInput")
    tables_d = nc.dram_tensor("tables", [F * CARD, D], fp16, kind="ExternalInput")
    ident_d = nc.dram_tensor("ident", [P, P], fp16, kind="ExternalInput")
    w1a_d = nc.dram_tensor("w1a", [DENSE + 1, H_BOT], fp16, kind="ExternalInput")
    w2a_d = nc.dram_tensor("w2a", [H_BOT + 1, D], fp16, kind="ExternalInput")
    tw1c_d = nc.dram_tensor("tw1c", [P, KC_N * H_TOP], fp16, kind="ExternalInput")
    tw1da_d = nc.dram_tensor("tw1da", [D + 1, H_TOP], fp16, kind="ExternalInput")
    tw2_d = nc.dram_tensor("tw2", [H_TOP, 1], fp16, kind="ExternalInput")
    tb2_d = nc.dram_tensor("tb2", [1], f32, kind="ExternalInput")
    y_d = nc.dram_tensor("y", [1, B_LOC], f32, kind="ExternalOutput")

    with tile.TileContext(nc) as tc:
        with (
            tc.tile_pool(name="const", bufs=1) as cpool,
            tc.tile_pool(name="embt", bufs=6) as embtp,
            tc.tile_pool(name="small", bufs=3) as smallp,
            tc.tile_pool(name="ptr", bufs=4, space="PSUM") as ptrp,
            tc.tile_pool(name="po1", bufs=2, space="PSUM") as po1p,
            tc.tile_pool(name="psmall", bufs=2, space="PSUM") as psmallp,
        ):
            # ---- index DMA first (contiguous, host-pre-transposed) ----
            comb_sb = cpool.tile([P, N_T * F], i32)
            nc.sync.dma_start(out=comb_sb[:], in_=comb_d[:, :])

            # ---- gather dispatches (gpsimd only): group 0 split for latency ----
            embs = [cpool.tile([P, TPG * K_EMB], fp16, name=f"emb{g}") for g in range(N_G)]
            spans = [(0, 0, 2 * F), (0, 2 * F, 4 * F)] + [
                (g, 0, TPG * F) for g in range(1, N_G)
            ]
            for g, f0, f1 in spans:
                nc.gpsimd.indirect_dma_start(
                    out=embs[g][:, f0 * D : f1 * D],
                    out_offset=None,
                    in_=tables_d[:, :],
                    in_offset=bass.IndirectOffsetOnAxis(
                        ap=comb_sb[:, g * TPG * F + f0 : g * TPG * F + f1], axis=0
                    ),
                )

            # ---- weights / constants (host-prepped fp16, all contiguous) ----
            dense_sb = cpool.tile([P, N_T * DENSE], fp16)
            nc.sync.dma_start(out=dense_sb[:], in_=dense_d[:, :])
            ident = cpool.tile([P, P], fp16)
            nc.sync.dma_start(out=ident[:], in_=ident_d[:, :])
            tw1_c = cpool.tile([P, KC_N * H_TOP], fp16)
            nc.sync.dma_start(out=tw1_c[:], in_=tw1c_d[:, :])
            tw1da_sb = cpool.tile([D + 1, H_TOP], fp16)
            nc.sync.dma_start(out=tw1da_sb[:], in_=tw1da_d[:, :])
            w1a_sb = cpool.tile([DENSE + 1, H_BOT], fp16)
            nc.sync.dma_start(out=w1a_sb[:], in_=w1a_d[:, :])
            w2a_sb = cpool.tile([H_BOT + 1, D], fp16)
            nc.sync.dma_start(out=w2a_sb[:], in_=w2a_d[:, :])
            tw2_sb = cpool.tile([H_TOP, 1], fp16)
            nc.sync.dma_start(out=tw2_sb[:], in_=tw2_d[:, :])
            tb2_sb = cpool.tile([1, 1], f32)
            nc.sync.dma_start(out=tb2_sb[:], in_=tb2_d[:, None])

            y_row = cpool.tile([1, B_LOC], f32)
            dta = [cpool.tile([D + 1, GROUP], fp16, name=f"dta{g}") for g in range(N_G)]

            # ---- bottom MLP for all groups up front (PE idles during the
            # first gather anyway); biases folded in via ones-rows ----
            def dense_tr(g):
                pdx_t = psmallp.tile([DENSE, GROUP], fp16, tag="psmall")
                for j in range(TPG):
                    t = g * TPG + j
                    nc.tensor.transpose(
                        out=pdx_t[:, bass.ts(j, P)],
                        in_=dense_sb[:, bass.ts(t, DENSE)],
                        identity=ident[:],
                    )
                return pdx_t

            def dense_mm1(g, pdx_t):
                dxt = smallp.tile([DENSE + 1, GROUP], fp16, tag="dxt")
                nc.vector.memset(dxt[:], 1.0)
                nc.vector.tensor_copy(out=dxt[0:DENSE, :], in_=pdx_t[:])
                ph = psmallp.tile([H_BOT, GROUP], f32, tag="psmall")
                nc.tensor.matmul(out=ph[:], lhsT=w1a_sb[:], rhs=dxt[:], start=True, stop=True)
                return ph

            def dense_mm2(g, ph):
                ha = smallp.tile([H_BOT + 1, GROUP], fp16, tag="ha")
                nc.vector.memset(ha[:], 1.0)
                nc.vector.tensor_scalar_max(ha[0:H_BOT, :], ph[:], 0.0)
                pd = psmallp.tile([D, GROUP], f32, tag="psmall")
                nc.tensor.matmul(out=pd[:], lhsT=w2a_sb[:], rhs=ha[:], start=True, stop=True)
                nc.vector.memset(dta[g][:], 1.0)
                nc.vector.tensor_copy(out=dta[g][0:D, :], in_=pd[:])

            pdx0 = dense_tr(0)
            pdx1 = dense_tr(1)
            ph0 = dense_mm1(0, pdx0)
            pdx2 = dense_tr(2)
            ph1 = dense_mm1(1, pdx1)
            dense_mm2(0, ph0)
            pdx3 = dense_tr(3)
            ph2 = dense_mm1(2, pdx2)
            dense_mm2(1, ph1)
            ph3 = dense_mm1(3, pdx3)
            dense_mm2(2, ph2)
            dense_mm2(3, ph3)

            # ---- embedding transposes + top-MLP accumulation ----
            tail = []  # deferred (o1 relu, tw2 matmul, sigmoid) of prev group

            def flush_tail():
                while tail:
                    tail.pop(0)()

            for g in range(N_G):
                po1 = po1p.tile([H_TOP, GROUP], f32, tag="po1")
                pending = []

                def emit_chunk(kc, g=g, po1=po1):
                    ptr_t = ptrp.tile([P, GROUP], fp16, tag="ptr")
                    for j in range(TPG):
                        nc.tensor.transpose(
                            out=ptr_t[:, bass.ts(j, P)],
                            in_=embs[g][:, j * K_EMB + kc * P : j * K_EMB + (kc + 1) * P],
                            identity=ident[:],
                        )
                    embt = embtp.tile([P, GROUP], fp16, tag="embt")
                    if kc % 2 == 0:
                        nc.vector.tensor_copy(out=embt[:], in_=ptr_t[:])
                    else:
                        nc.scalar.activation(
                            out=embt[:], in_=ptr_t[:],
                            func=mybir.ActivationFunctionType.Copy,
                        )

                    def mm():
                        nc.tensor.matmul(
                            out=po1[:],
                            lhsT=tw1_c[:, bass.ts(kc, H_TOP)],
                            rhs=embt[:],
                            start=(kc == 0),
                            stop=False,
                        )
                    pending.append(mm)

                # pairs of chunks: 8 transposes, then the 2 matmuls two
                # pairs behind (casts get ~2 pair-slots of slack)
                for pair in range(7):  # pairs: (0,1) (2,3) ... (10,11) (12,)
                    emit_chunk(2 * pair)
                    if 2 * pair + 1 < KC_N:
                        emit_chunk(2 * pair + 1)
                    if pair == 0:
                        flush_tail()  # prev group's tw2 matmul + sigmoid
                    if pair >= 2:
                        pending.pop(0)()
                        if pending:
                            pending.pop(0)()
                while pending:
                    pending.pop(0)()
                nc.tensor.matmul(
                    out=po1[:], lhsT=tw1da_sb[:], rhs=dta[g][:], start=False, stop=True
                )

                def make_tail(g=g, po1=po1):
                    def run():
                        o1 = smallp.tile([H_TOP, GROUP], fp16, tag="o1")
                        nc.vector.tensor_scalar_max(o1[:], po1[:], 0.0)
                        plg = psmallp.tile([1, GROUP], f32, tag="psmall")
                        nc.tensor.matmul(
                            out=plg[:], lhsT=tw2_sb[:], rhs=o1[:], start=True, stop=True
                        )
                        nc.scalar.activation(
                            out=y_row[:, bass.ts(g, GROUP)],
                            in_=plg[:],
                            func=mybir.ActivationFunctionType.Sigmoid,
                            bias=tb2_sb[:],
                        )
                    return run

                tail.append(make_tail())
            flush_tail()

            nc.sync.dma_start(out=y_d[:, :], in_=y_row[:])

    nc.compile()
    return nc


_NC_CACHE = {}


def _get_nc():
    if "nc" not in _NC_CACHE:
        _NC_CACHE["nc"] = build_kernel()
    return _NC_CACHE["nc"]


def make_in_maps(dense_x, sparse_x, tables, w1, b1, w2, b2, tw1, tb1, tw2, tb2):
    tables_flat = np.ascontiguousarray(
        np.asarray(tables).reshape(F * CARD, D).astype(np.float16)
    )
    comb = np.asarray(sparse_x).astype(np.int32) + (
        np.arange(F, dtype=np.int32) * CARD
    )[None, :]
    dense_f = np.asarray(dense_x).astype(np.float16)
    tw1 = np.asarray(tw1, np.float32)
    # tw1c[p, (c m)] = tw1[c*128+p, m]
    tw1c = np.ascontiguousarray(
        tw1[:K_EMB].astype(np.float16).reshape(KC_N, P, H_TOP).transpose(1, 0, 2).reshape(P, KC_N * H_TOP)
    )
    shared = {
        "tables": tables_flat,
        "ident": np.eye(P, dtype=np.float16),
        "w1a": np.vstack([np.asarray(w1, np.float32), np.asarray(b1, np.float32)[None, :]]).astype(np.float16),
        "w2a": np.vstack([np.asarray(w2, np.float32), np.asarray(b2, np.float32)[None, :]]).astype(np.float16),
        "tw1c": tw1c,
        "tw1da": np.vstack([tw1[K_EMB : K_EMB + D], np.asarray(tb1, np.float32)[None, :]]).astype(np.float16),
        "tw2": np.asarray(tw2, np.float32).astype(np.float16),
        "tb2": np.ascontiguousarray(np.asarray(tb2, np.float32)),
    }
    in_maps = []
    for c in range(N_CORES):
        m = dict(shared)
        # host pre-transpose: [p, (t f)] so the device DMA is contiguous
        dl = dense_f[c * B_LOC : (c + 1) * B_LOC]
        m["dense_x"] = np.ascontiguousarray(
            dl.reshape(N_T, P, DENSE).transpose(1, 0, 2).reshape(P, N_T * DENSE)
        )
        cl = comb[c * B_LOC : (c + 1) * B_LOC]
        m["sparse_x"] = np.ascontiguousarray(
            cl.reshape(N_T, P, F).transpose(1, 0, 2).reshape(P, N_T * F)
        )
        in_maps.append(m)
    return in_maps


def kernel(**inputs):
    from concourse.bass_utils import run_bass_kernel_spmd

    nc = _get_nc()
    in_maps = make_in_maps(**inputs)
    res = run_bass_kernel_spmd(nc, in_maps, core_ids=list(range(N_CORES)))
    out = np.concatenate([r["y"].reshape(-1) for r in res.results])
    return out.reshape(B, 1).astype(np.float32)


# revision 9
# speedup vs baseline: 1.6767x; 1.3316x over previous
"""DLRM forward (embedding gather + tiny MLPs) as a Bass/Tile kernel on 8 trn2 cores.

Sharding: data-parallel over the batch (tables replicated in each core's HBM —
total gather traffic is the same as model-parallel but needs no collectives).
Each core computes 2048 rows end-to-end and returns [1, 2048] sigmoid outputs.

v4 (v3 62.5µs, v1 68.7µs, v0 110µs):
  - embedding tables quantized host-side to fp8e4m3 (scaled x16 into the
    format's normal range; numpy-validated rel-err contribution 1.2e-4 —
    the embedding term is small vs the dense path in this model).  Gather
    traffic halves to 64B/row.
  - fp8 pairs are bit-packed as uint16: tables are staged as [V, 32] u16,
    so one PE transpose of a [128, 128] u16 chunk moves TWO fp8 k-values
    per lane — 28 transposes + 7 casts per group instead of 52 + 13.
    uint16 survives the fp32-internal engines exactly; an integer identity
    keeps the PE transpose bit-exact.
  - the transposed pair layout (k-pair adjacent per partition) is exactly
    DoubleRow fp8 matmul's operand shape: 7 DoubleRow matmuls per group
    contract 256 k each (0.5 cycles/row), lhsT = host-packed tw1 pairs
    [p, i, m], rhs = the u16 tile bitcast to fp8 [p, i(step1), n(step2)].
  - x16(tables) * x16(tw1) = x256 accumulator scale; the dense-path
    weights (tw1da incl. folded tb1) are scaled x256 in fp16, and the
    group relu is a fused (x 1/256, max 0) tensor_scalar on DVE.
  - dense bottom-MLP: group 0 up front (fills the first-gather wait),
    groups 1-3 interleaved into their group's PE stream; ones-rows live
    in persistent ping-pong tiles memset once at startup.
"""

import numpy as np
import ml_dtypes

import concourse.bass as bass
import concourse.mybir as mybir
import concourse.tile as tile
from concourse import bacc

P = 128

N_CORES = 8
B = 16384
F = 26
D = 64
DENSE = 13
CARD = 100000
H_BOT = 8
H_TOP = 16

f32 = mybir.dt.float32
i32 = mybir.dt.int32
fp16 = mybir.dt.float16
fp8 = mybir.dt.float8e4
u16 = mybir.dt.uint16
bf16 = mybir.dt.bfloat16

B_LOC = B // N_CORES          # 2048
K_EMB = F * D                 # 1664
GROUP = 512
TPG = GROUP // P              # 4
N_G = B_LOC // GROUP          # 4
N_T = B_LOC // P              # 16

W_T = F * D // 2              # 832 u16 per sample-tile row
C_N = 7                       # u16 chunks of <=128: 6*128 + 64
CW = [128] * 6 + [64]
C_OFF = [0, 128, 256, 384, 512, 640, 768]

FP8_SCALE = 16.0              # tables and tw1 each scaled by this


def build_kernel():
    nc = bacc.Bacc("TRN2", target_bir_lowering=False)
    comb_d = nc.dram_tensor("sparse_x", [P, N_T * F], i32, kind="ExternalInput")
    dense_d = nc.dram_tensor("dense_x", [P, N_T * DENSE], fp16, kind="ExternalInput")
    tables_d = nc.dram_tensor("tables", [F * CARD, D // 2], u16, kind="ExternalInput")
    ident_d = nc.dram_tensor("ident", [P, P], fp16, kind="ExternalInput")
    identu_d = nc.dram_tensor("identu", [P, P], bf16, kind="ExternalInput")
    w1a_d = nc.dram_tensor("w1a", [DENSE + 1, H_BOT], fp16, kind="ExternalInput")
    w2a_d = nc.dram_tensor("w2a", [H_BOT + 1, D], fp16, kind="ExternalInput")
    tw1dr_d = nc.dram_tensor("tw1dr", [P, C_N * 2 * H_TOP], fp8, kind="ExternalInput")
    tw1da_d = nc.dram_tensor("tw1da", [D + 1, H_TOP], fp16, kind="ExternalInput")
    tw2_d = nc.dram_tensor("tw2", [H_TOP, 1], fp16, kind="ExternalInput")
    tb2_d = nc.dram_tensor("tb2", [1], f32, kind="ExternalInput")
    y_d = nc.dram_tensor("y", [1, B_LOC], f32, kind="ExternalOutput")

    with tile.TileContext(nc) as tc:
        with (
            tc.tile_pool(name="const", bufs=1) as cpool,
            tc.tile_pool(name="embt", bufs=5) as embtp,
            tc.tile_pool(name="small", bufs=2) as smallp,
            tc.tile_pool(name="ptr", bufs=3, space="PSUM") as ptrp,
            tc.tile_pool(name="po1", bufs=2, space="PSUM") as po1p,
            tc.tile_pool(name="psmall", bufs=2, space="PSUM") as psmallp,
        ):
            # ---- index DMA first (contiguous, host-pre-transposed) ----
            comb_sb = cpool.tile([P, N_T * F], i32)
            nc.sync.dma_start(out=comb_sb[:], in_=comb_d[:, :])

            # ---- gather dispatches (gpsimd only): group 0 split for latency ----
            embs = [cpool.tile([P, TPG * W_T], u16, name=f"emb{g}") for g in range(N_G)]
            spans = [(0, 0, 2 * F), (0, 2 * F, 4 * F)] + [
                (g, 0, TPG * F) for g in range(1, N_G)
            ]
            for g, f0, f1 in spans:
                nc.gpsimd.indirect_dma_start(
                    out=embs[g][:, f0 * (D // 2) : f1 * (D // 2)],
                    out_offset=None,
                    in_=tables_d[:, :],
                    in_offset=bass.IndirectOffsetOnAxis(
                        ap=comb_sb[:, g * TPG * F + f0 : g * TPG * F + f1], axis=0
                    ),
                )

            # ---- weights / constants (host-prepped, all contiguous) ----
            dense_sb = cpool.tile([P, N_T * DENSE], fp16)
            nc.sync.dma_start(out=dense_sb[:], in_=dense_d[:, :])
            ident = cpool.tile([P, P], fp16)
            nc.sync.dma_start(out=ident[:], in_=ident_d[:, :])
            identu = cpool.tile([P, P], bf16)
            nc.sync.dma_start(out=identu[:], in_=identu_d[:, :])
            tw1dr_sb = cpool.tile([P, C_N * 2 * H_TOP], fp8)
            nc.sync.dma_start(out=tw1dr_sb[:], in_=tw1dr_d[:, :])
            tw1da_sb = cpool.tile([D + 1, H_TOP], fp16)
            nc.sync.dma_start(out=tw1da_sb[:], in_=tw1da_d[:, :])
            w1a_sb = cpool.tile([DENSE + 1, H_BOT], fp16)
            nc.sync.dma_start(out=w1a_sb[:], in_=w1a_d[:, :])
            w2a_sb = cpool.tile([H_BOT + 1, D], fp16)
            nc.sync.dma_start(out=w2a_sb[:], in_=w2a_d[:, :])
            tw2_sb = cpool.tile([H_TOP, 1], fp16)
            nc.sync.dma_start(out=tw2_sb[:], in_=tw2_d[:, :])
            tb2_sb = cpool.tile([1, 1], f32)
            nc.sync.dma_start(out=tb2_sb[:], in_=tb2_d[:, None])

            y_row = cpool.tile([1, B_LOC], f32)
            dta = [cpool.tile([D + 1, GROUP], fp16, name=f"dta{g}") for g in range(N_G)]
            dxt2 = [cpool.tile([DENSE + 1, GROUP], fp16, name=f"dxt{i}") for i in range(2)]
            ha2 = [cpool.tile([H_BOT + 1, GROUP], fp16, name=f"ha{i}") for i in range(2)]
            # ones-rows: memset whole tiles once; data rows overwritten each use
            for t in dxt2 + ha2 + dta:
                nc.vector.memset(t[:], 1.0)

            # ---- dense bottom-MLP pieces (biases folded via ones-rows) ----
            def dense_tr(g):
                pdx_t = psmallp.tile([DENSE, GROUP], fp16, tag="psmall")
                for j in range(TPG):
                    t = g * TPG + j
                    nc.tensor.transpose(
                        out=pdx_t[:, bass.ts(j, P)],
                        in_=dense_sb[:, bass.ts(t, DENSE)],
                        identity=ident[:],
                    )
                return pdx_t

            def dense_mm1(g, pdx_t):
                dxt = dxt2[g % 2]
                nc.vector.tensor_copy(out=dxt[0:DENSE, :], in_=pdx_t[:])
                ph = psmallp.tile([H_BOT, GROUP], f32, tag="psmall")
                nc.tensor.matmul(out=ph[:], lhsT=w1a_sb[:], rhs=dxt[:], start=True, stop=True)
                return ph

            def dense_mm2(g, ph):
                ha = ha2[g % 2]
                nc.vector.tensor_scalar_max(ha[0:H_BOT, :], ph[:], 0.0)
                pd = psmallp.tile([D, GROUP], f32, tag="psmall")
                nc.tensor.matmul(out=pd[:], lhsT=w2a_sb[:], rhs=ha[:], start=True, stop=True)
                nc.vector.tensor_copy(out=dta[g][0:D, :], in_=pd[:])

            # group 0 dense runs up front, under the first gather
            pdx0 = dense_tr(0)
            ph0 = dense_mm1(0, pdx0)
            dense_mm2(0, ph0)

            # ---- fp8-pair transposes + DoubleRow top-MLP accumulation ----
            tail = []  # deferred (o1 relu, tw2 matmul, sigmoid) of prev group

            def flush_tail():
                while tail:
                    tail.pop(0)()

            for g in range(N_G):
                po1 = po1p.tile([H_TOP, GROUP], f32, tag="po1")
                pending = []
                dense_state = {}

                def emit_chunk(c, g=g, po1=po1, pending=pending):
                    cw = CW[c]
                    ptr_t = ptrp.tile([P, GROUP], bf16, tag="ptr")
                    for j in range(TPG):
                        o = j * W_T + C_OFF[c]
                        nc.tensor.transpose(
                            out=ptr_t[0:cw, bass.ts(j, P)],
                            in_=embs[g][:, o : o + cw].bitcast(bf16),
                            identity=identu[:],
                        )
                    embt = embtp.tile([P, GROUP], bf16, tag="embt")
                    if c % 2 == 0:
                        nc.vector.tensor_copy(out=embt[0:cw, :], in_=ptr_t[0:cw, :])
                    else:
                        nc.scalar.activation(
                            out=embt[0:cw, :], in_=ptr_t[0:cw, :],
                            func=mybir.ActivationFunctionType.Copy,
                        )

                    def mm():
                        lhsT = tw1dr_sb[0:cw, c * 32 : (c + 1) * 32].rearrange(
                            "p (i m) -> p i m", i=2
                        )
                        rhs = embt[0:cw, :].bitcast(fp8).rearrange(
                            "p (n i) -> p i n", i=2
                        )
                        nc.tensor.matmul(
                            out=po1[:],
                            lhsT=lhsT,
                            rhs=rhs,
                            start=(c == 0),
                            stop=False,
                            perf_mode=mybir.MatmulPerfMode.DoubleRow,
                        )
                    pending.append(mm)

                emit_chunk(0)
                if g > 0:
                    dense_state["pdx"] = dense_tr(g)
                flush_tail()
                emit_chunk(1)
                emit_chunk(2)
                pending.pop(0)()                      # MM(0)
                if g > 0:
                    dense_state["ph"] = dense_mm1(g, dense_state["pdx"])
                emit_chunk(3)
                pending.pop(0)()                      # MM(1)
                emit_chunk(4)
                pending.pop(0)()                      # MM(2)
                if g > 0:
                    dense_mm2(g, dense_state["ph"])
                emit_chunk(5)
                pending.pop(0)()                      # MM(3)
                emit_chunk(6)
                while pending:
                    pending.pop(0)()                  # MM(4..6)
                nc.tensor.matmul(
                    out=po1[:], lhsT=tw1da_sb[:], rhs=dta[g][:], start=False, stop=True
                )

                def make_tail(g=g, po1=po1):
                    def run():
                        o1 = smallp.tile([H_TOP, GROUP], fp16, tag="o1")
                        nc.vector.tensor_scalar(
                            out=o1[:], in0=po1[:],
                            scalar1=1.0 / (FP8_SCALE * FP8_SCALE), scalar2=0.0,
                            op0=mybir.AluOpType.mult, op1=mybir.AluOpType.max,
                        )
                        plg = psmallp.tile([1, GROUP], f32, tag="psmall")
                        nc.tensor.matmul(
                            out=plg[:], lhsT=tw2_sb[:], rhs=o1[:], start=True, stop=True
                        )
                        nc.scalar.activation(
                            out=y_row[:, bass.ts(g, GROUP)],
                            in_=plg[:],
                            func=mybir.ActivationFunctionType.Sigmoid,
                            bias=tb2_sb[:],
                        )
                    return run

                tail.append(make_tail())
            flush_tail()

            nc.sync.dma_start(out=y_d[:, :], in_=y_row[:])

    nc.compile()
    return nc


_NC_CACHE = {}


def _get_nc():
    if "nc" not in _NC_CACHE:
        _NC_CACHE["nc"] = build_kernel()
    return _NC_CACHE["nc"]


FP8_NP = ml_dtypes.float8_e4m3  # what mybir.dt.float8e4 maps to


def make_in_maps(dense_x, sparse_x, tables, w1, b1, w2, b2, tw1, tb1, tw2, tb2):
    s = FP8_SCALE
    t8 = (np.asarray(tables, np.float32).reshape(F * CARD, D) * s).astype(FP8_NP)
    tables_u16 = np.ascontiguousarray(t8).view(np.uint16)  # [V, 32]
    comb = np.asarray(sparse_x).astype(np.int32) + (
        np.arange(F, dtype=np.int32) * CARD
    )[None, :]
    dense_f = np.asarray(dense_x).astype(np.float16)
    tw1 = np.asarray(tw1, np.float32)
    # tw1dr[p, c*32 + i*16 + m] = fp8(s * tw1[c*256 + 2p + i, m])
    tw1dr = np.zeros((P, C_N * 2 * H_TOP), dtype=FP8_NP)
    for c in range(C_N):
        rows = 2 * CW[c]
        blk = (tw1[c * 256 : c * 256 + rows] * s).astype(FP8_NP)  # [rows, 16]
        blk = blk.reshape(CW[c], 2, H_TOP).reshape(CW[c], 2 * H_TOP)
        tw1dr[0 : CW[c], c * 32 : (c + 1) * 32] = blk
    tw1da = np.vstack(
        [tw1[K_EMB : K_EMB + D], np.asarray(tb1, np.float32)[None, :]]
    ) * (s * s)
    shared = {
        "tables": tables_u16,
        "ident": np.eye(P, dtype=np.float16),
        "identu": np.eye(P, dtype=ml_dtypes.bfloat16),
        "w1a": np.vstack([np.asarray(w1, np.float32), np.asarray(b1, np.float32)[None, :]]).astype(np.float16),
        "w2a": np.vstack([np.asarray(w2, np.float32), np.asarray(b2, np.float32)[None, :]]).astype(np.float16),
        "tw1dr": tw1dr,
        "tw1da": tw1da.astype(np.float16),
        "tw2": np.asarray(tw2, np.float32).astype(np.float16),
        "tb2": np.ascontiguousarray(np.asarray(tb2, np.float32)),
    }
    in_maps = []
    for c in range(N_CORES):
        m = dict(shared)
        # host pre-transpose: [p, (t f)] so the device DMA is contiguous
        dl = dense_f[c * B_LOC : (c + 1) * B_LOC]
        m["dense_x"] = np.ascontiguousarray(
            dl.reshape(N_T, P, DENSE).transpose(1, 0, 2).reshape(P, N_T * DENSE)
        )
        cl = comb[c * B_LOC : (c + 1) * B_LOC]
        m["sparse_x"] = np.ascontiguousarray(
            cl.reshape(N_T, P, F).transpose(1, 0, 2).reshape(P, N_T * F)
        )
        in_maps.append(m)
    return in_maps


def kernel(**inputs):
    from concourse.bass_utils import run_bass_kernel_spmd

    nc = _get_nc()
    in_maps = make_in_maps(**inputs)
    res = run_bass_kernel_spmd(nc, in_maps, core_ids=list(range(N_CORES)))
    out = np.concatenate([r["y"].reshape(-1) for r in res.results])
    return out.reshape(B, 1).astype(np.float32)


# revision 10
# speedup vs baseline: 1.7772x; 1.0600x over previous
"""DLRM forward (embedding gather + tiny MLPs) as a Bass/Tile kernel on 8 trn2 cores.

Sharding: data-parallel over the batch (tables replicated in each core's HBM —
total gather traffic is the same as model-parallel but needs no collectives).
Each core computes 2048 rows end-to-end and returns [1, 2048] sigmoid outputs.

v5 (v4 46.9µs, v3 62.5µs, v0 110µs):
  - fp8e4m3 tables (x16 scale) bit-packed as u16 pairs; PE transposes move
    the pairs typed as bf16 (our fp8 bytes can never form bf16 NaN/Inf
    patterns), DoubleRow fp8 matmuls contract 256 k per pass.
  - weight DMAs split across the sync AND scalar HWDGE queues ordered by
    need-time: v4 lost ~3µs because one sequencer issued all DMAs serially
    and the gather + first dense matmul waited behind them.
  - dense_x gets its ones-column on the host, so the bottom-MLP bias row
    arrives via the transpose and the dxt staging needs no memset.
  - transposes/casts work on double-chunks: one [128, 1024] PSUM tile per
    two k-chunks, ONE cast instruction each (sequencer relief for DVE,
    which was 87% busy issuing event-semaphores in v4).
  - per-group y DMA; last group's relu/tw2/sigmoid tail split in halves
    to pipeline the drain.
"""

import numpy as np
import ml_dtypes

import concourse.bass as bass
import concourse.mybir as mybir
import concourse.tile as tile
from concourse import bacc

P = 128

N_CORES = 8
B = 16384
F = 26
D = 64
DENSE = 13
DENSE_A = DENSE + 1           # host-appended ones column
CARD = 100000
H_BOT = 8
H_TOP = 16

f32 = mybir.dt.float32
i32 = mybir.dt.int32
fp16 = mybir.dt.float16
fp8 = mybir.dt.float8e4
u16 = mybir.dt.uint16
bf16 = mybir.dt.bfloat16

B_LOC = B // N_CORES          # 2048
K_EMB = F * D                 # 1664
GROUP = 512
TPG = GROUP // P              # 4
N_G = B_LOC // GROUP          # 4
N_T = B_LOC // P              # 16

W_T = F * D // 2              # 832 u16 per sample-tile row
CW = [128] * 6 + [64]         # u16 chunk widths (pairs of fp8 k-values)

FP8_SCALE = 16.0              # tables and tw1 each scaled by this


def build_kernel():
    nc = bacc.Bacc("TRN2", target_bir_lowering=False)
    comb_d = nc.dram_tensor("sparse_x", [P, N_T * F], i32, kind="ExternalInput")
    dense_d = nc.dram_tensor("dense_x", [P, N_T * DENSE_A], fp16, kind="ExternalInput")
    tables_d = nc.dram_tensor("tables", [F * CARD, D // 2], u16, kind="ExternalInput")
    ident_d = nc.dram_tensor("ident", [P, P], fp16, kind="ExternalInput")
    identu_d = nc.dram_tensor("identu", [P, P], bf16, kind="ExternalInput")
    w1a_d = nc.dram_tensor("w1a", [DENSE_A, H_BOT], fp16, kind="ExternalInput")
    w2a_d = nc.dram_tensor("w2a", [H_BOT + 1, D], fp16, kind="ExternalInput")
    tw1dr_d = nc.dram_tensor("tw1dr", [P, 7 * 2 * H_TOP], fp8, kind="ExternalInput")
    tw1da_d = nc.dram_tensor("tw1da", [D + 1, H_TOP], fp16, kind="ExternalInput")
    tw2_d = nc.dram_tensor("tw2", [H_TOP, 1], fp16, kind="ExternalInput")
    tb2_d = nc.dram_tensor("tb2", [1], f32, kind="ExternalInput")
    y_d = nc.dram_tensor("y", [1, B_LOC], f32, kind="ExternalOutput")

    with tile.TileContext(nc) as tc:
        with (
            tc.tile_pool(name="const", bufs=1) as cpool,
            tc.tile_pool(name="embt", bufs=3) as embtp,
            tc.tile_pool(name="small", bufs=3) as smallp,
            tc.tile_pool(name="ptr", bufs=3, space="PSUM") as ptrp,
            tc.tile_pool(name="po1", bufs=2, space="PSUM") as po1p,
            tc.tile_pool(name="psmall", bufs=2, space="PSUM") as psmallp,
        ):
            # ---- index DMA first: the gather dispatches wait only on this ----
            comb_sb = cpool.tile([P, N_T * F], i32)
            nc.sync.dma_start(out=comb_sb[:], in_=comb_d[:, :])

            # ---- gather dispatches (gpsimd only): group 0 split for latency ----
            embs = [cpool.tile([P, TPG * W_T], u16, name=f"emb{g}") for g in range(N_G)]
            spans = [(0, 0, 2 * F), (0, 2 * F, 4 * F)] + [
                (g, 0, TPG * F) for g in range(1, N_G)
            ]
            for g, f0, f1 in spans:
                nc.gpsimd.indirect_dma_start(
                    out=embs[g][:, f0 * (D // 2) : f1 * (D // 2)],
                    out_offset=None,
                    in_=tables_d[:, :],
                    in_offset=bass.IndirectOffsetOnAxis(
                        ap=comb_sb[:, g * TPG * F + f0 : g * TPG * F + f1], axis=0
                    ),
                )

            # ---- weights (host-prepped, contiguous), split across the two
            # HWDGE queues in need-order so nothing queues behind a big DMA ----
            dense_sb = cpool.tile([P, N_T * DENSE_A], fp16)
            nc.sync.dma_start(out=dense_sb[:], in_=dense_d[:, :])
            ident = cpool.tile([P, P], fp16)
            nc.sync.dma_start(out=ident[:], in_=ident_d[:, :])
            w1a_sb = cpool.tile([DENSE_A, H_BOT], fp16)
            nc.scalar.dma_start(out=w1a_sb[:], in_=w1a_d[:, :])
            identu = cpool.tile([P, P], bf16)
            nc.scalar.dma_start(out=identu[:], in_=identu_d[:, :])
            w2a_sb = cpool.tile([H_BOT + 1, D], fp16)
            nc.scalar.dma_start(out=w2a_sb[:], in_=w2a_d[:, :])
            tw1dr_sb = cpool.tile([P, 7 * 2 * H_TOP], fp8)
            nc.scalar.dma_start(out=tw1dr_sb[:], in_=tw1dr_d[:, :])
            tw1da_sb = cpool.tile([D + 1, H_TOP], fp16)
            nc.scalar.dma_start(out=tw1da_sb[:], in_=tw1da_d[:, :])
            tw2_sb = cpool.tile([H_TOP, 1], fp16)
            nc.sync.dma_start(out=tw2_sb[:], in_=tw2_d[:, :])
            tb2_sb = cpool.tile([1, 1], f32)
            nc.sync.dma_start(out=tb2_sb[:], in_=tb2_d[:, None])

            y_row = cpool.tile([1, B_LOC], f32)
            dta = [cpool.tile([D + 1, GROUP], fp16, name=f"dta{g}") for g in range(N_G)]
            ha2 = [cpool.tile([H_BOT + 1, GROUP], fp16, name=f"ha{i}") for i in range(2)]
            # ones-rows: memset once; data rows overwritten per use. dta[1:]
            # memset on gpsimd after the dispatches (not needed until later).
            for t in ha2 + dta[:1]:
                nc.vector.memset(t[:], 1.0)
            for t in dta[1:]:
                nc.gpsimd.memset(t[:], 1.0)

            # ---- dense bottom-MLP pieces (biases folded via ones-rows;
            # dxt's ones row comes in with the host data) ----
            def dense_tr(g):
                pdx_t = psmallp.tile([DENSE_A, GROUP], fp16, tag="psmall")
                for j in range(TPG):
                    t = g * TPG + j
                    nc.tensor.transpose(
                        out=pdx_t[:, bass.ts(j, P)],
                        in_=dense_sb[:, bass.ts(t, DENSE_A)],
                        identity=ident[:],
                    )
                return pdx_t

            def dense_mm1(g, pdx_t):
                dxt = smallp.tile([DENSE_A, GROUP], fp16, tag="dxt")
                nc.vector.tensor_copy(out=dxt[:], in_=pdx_t[:])
                ph = psmallp.tile([H_BOT, GROUP], f32, tag="psmall")
                nc.tensor.matmul(out=ph[:], lhsT=w1a_sb[:], rhs=dxt[:], start=True, stop=True)
                return ph

            def dense_mm2(g, ph):
                ha = ha2[g % 2]
                nc.vector.tensor_scalar_max(ha[0:H_BOT, :], ph[:], 0.0)
                pd = psmallp.tile([D, GROUP], f32, tag="psmall")
                nc.tensor.matmul(out=pd[:], lhsT=w2a_sb[:], rhs=ha[:], start=True, stop=True)
                nc.vector.tensor_copy(out=dta[g][0:D, :], in_=pd[:])

            # group 0 dense runs up front, under the first gather
            pdx0 = dense_tr(0)
            ph0 = dense_mm1(0, pdx0)
            dense_mm2(0, ph0)

            # ---- fp8-pair transposes + DoubleRow top-MLP accumulation ----
            tail = []  # deferred (o1 relu, tw2 matmul, sigmoid) of prev group

            def flush_tail():
                while tail:
                    tail.pop(0)()

            for g in range(N_G):
                po1 = po1p.tile([H_TOP, GROUP], f32, tag="po1")
                mms = []

                def emit_dchunk(dc, g=g, po1=po1, mms=mms):
                    # two k-chunks (2dc, 2dc+1) -> one PSUM tile, one cast
                    cs = [2 * dc] + ([2 * dc + 1] if 2 * dc + 1 < 7 else [])
                    ptr_t = ptrp.tile([P, 2 * GROUP], bf16, tag="ptr")
                    for ci, c in enumerate(cs):
                        cw = CW[c]
                        for j in range(TPG):
                            o = j * W_T + c * 128
                            nc.tensor.transpose(
                                out=ptr_t[0:cw, ci * GROUP + j * P : ci * GROUP + (j + 1) * P],
                                in_=embs[g][:, o : o + cw].bitcast(bf16),
                                identity=identu[:],
                            )
                    embt = embtp.tile([P, 2 * GROUP], bf16, tag="embt")
                    wid = len(cs) * GROUP
                    if dc % 2 == 0:
                        nc.vector.tensor_copy(out=embt[:, 0:wid], in_=ptr_t[:, 0:wid])
                    else:
                        nc.scalar.activation(
                            out=embt[:, 0:wid], in_=ptr_t[:, 0:wid],
                            func=mybir.ActivationFunctionType.Copy,
                        )

                    for ci, c in enumerate(cs):
                        def mm(c=c, ci=ci, embt=embt):
                            cw = CW[c]
                            lhsT = tw1dr_sb[0:cw, c * 32 : (c + 1) * 32].rearrange(
                                "p (i m) -> p i m", i=2
                            )
                            rhs = embt[0:cw, bass.ts(ci, GROUP)].bitcast(fp8).rearrange(
                                "p (n i) -> p i n", i=2
                            )
                            nc.tensor.matmul(
                                out=po1[:], lhsT=lhsT, rhs=rhs,
                                start=(c == 0), stop=False,
                                perf_mode=mybir.MatmulPerfMode.DoubleRow,
                            )
                        mms.append(mm)

                emit_dchunk(0)
                if g > 0:
                    pdx_t = dense_tr(g)
                flush_tail()
                emit_dchunk(1)
                mms.pop(0)()                      # MM(0)
                if g > 0:
                    ph = dense_mm1(g, pdx_t)
                mms.pop(0)()                      # MM(1)
                emit_dchunk(2)
                mms.pop(0)()                      # MM(2)
                if g > 0:
                    dense_mm2(g, ph)
                mms.pop(0)()                      # MM(3)
                emit_dchunk(3)                    # single chunk 6
                mms.pop(0)()                      # MM(4)
                mms.pop(0)()                      # MM(5)
                mms.pop(0)()                      # MM(6)
                nc.tensor.matmul(
                    out=po1[:], lhsT=tw1da_sb[:], rhs=dta[g][:], start=False, stop=True
                )

                def make_tail(g=g, po1=po1):
                    halves = 2 if g == N_G - 1 else 1
                    def run():
                        o1 = smallp.tile([H_TOP, GROUP], fp16, tag="o1")
                        w = GROUP // halves
                        plgs = []
                        for h in range(halves):
                            nc.vector.tensor_scalar(
                                out=o1[:, h * w : (h + 1) * w],
                                in0=po1[:, h * w : (h + 1) * w],
                                scalar1=1.0 / (FP8_SCALE * FP8_SCALE), scalar2=0.0,
                                op0=mybir.AluOpType.mult, op1=mybir.AluOpType.max,
                            )
                            plg = psmallp.tile([1, GROUP], f32, tag="psmall")
                            nc.tensor.matmul(
                                out=plg[:, 0:w], lhsT=tw2_sb[:],
                                rhs=o1[:, h * w : (h + 1) * w], start=True, stop=True,
                            )
                            plgs.append(plg)
                        for h, plg in enumerate(plgs):
                            nc.scalar.activation(
                                out=y_row[:, g * GROUP + h * w : g * GROUP + (h + 1) * w],
                                in_=plg[:, 0:w],
                                func=mybir.ActivationFunctionType.Sigmoid,
                                bias=tb2_sb[:],
                            )
                        nc.sync.dma_start(
                            out=y_d[:, bass.ts(g, GROUP)],
                            in_=y_row[:, bass.ts(g, GROUP)],
                        )
                    return run

                tail.append(make_tail())
            flush_tail()

    nc.compile()
    return nc


_NC_CACHE = {}


def _get_nc():
    if "nc" not in _NC_CACHE:
        _NC_CACHE["nc"] = build_kernel()
    return _NC_CACHE["nc"]


FP8_NP = ml_dtypes.float8_e4m3  # what mybir.dt.float8e4 maps to


def make_in_maps(dense_x, sparse_x, tables, w1, b1, w2, b2, tw1, tb1, tw2, tb2):
    s = FP8_SCALE
    t8 = (np.asarray(tables, np.float32).reshape(F * CARD, D) * s).astype(FP8_NP)
    tables_u16 = np.ascontiguousarray(t8).view(np.uint16)  # [V, 32]
    comb = np.asarray(sparse_x).astype(np.int32) + (
        np.arange(F, dtype=np.int32) * CARD
    )[None, :]
    dense_f = np.asarray(dense_x).astype(np.float16)
    dense_aug = np.concatenate(
        [dense_f, np.ones((dense_f.shape[0], 1), np.float16)], axis=1
    )  # [B, 14] with ones column
    tw1 = np.asarray(tw1, np.float32)
    # tw1dr[p, c*32 + i*16 + m] = fp8(s * tw1[c*256 + 2p + i, m])
    tw1dr = np.zeros((P, 7 * 2 * H_TOP), dtype=FP8_NP)
    for c in range(7):
        rows = 2 * CW[c]
        blk = (tw1[c * 256 : c * 256 + rows] * s).astype(FP8_NP)  # [rows, 16]
        blk = blk.reshape(CW[c], 2, H_TOP).reshape(CW[c], 2 * H_TOP)
        tw1dr[0 : CW[c], c * 32 : (c + 1) * 32] = blk
    tw1da = np.vstack(
        [tw1[K_EMB : K_EMB + D], np.asarray(tb1, np.float32)[None, :]]
    ) * (s * s)
    shared = {
        "tables": tables_u16,
        "ident": np.eye(P, dtype=np.float16),
        "identu": np.eye(P, dtype=ml_dtypes.bfloat16),
        "w1a": np.vstack([np.asarray(w1, np.float32), np.asarray(b1, np.float32)[None, :]]).astype(np.float16),
        "w2a": np.vstack([np.asarray(w2, np.float32), np.asarray(b2, np.float32)[None, :]]).astype(np.float16),
        "tw1dr": tw1dr,
        "tw1da": tw1da.astype(np.float16),
        "tw2": np.asarray(tw2, np.float32).astype(np.float16),
        "tb2": np.ascontiguousarray(np.asarray(tb2, np.float32)),
    }
    in_maps = []
    for c in range(N_CORES):
        m = dict(shared)
        # host pre-transpose: [p, (t f)] so the device DMA is contiguous
        dl = dense_aug[c * B_LOC : (c + 1) * B_LOC]
        m["dense_x"] = np.ascontiguousarray(
            dl.reshape(N_T, P, DENSE_A).transpose(1, 0, 2).reshape(P, N_T * DENSE_A)
        )
        cl = comb[c * B_LOC : (c + 1) * B_LOC]
        m["sparse_x"] = np.ascontiguousarray(
            cl.reshape(N_T, P, F).transpose(1, 0, 2).reshape(P, N_T * F)
        )
        in_maps.append(m)
    return in_maps


def kernel(**inputs):
    from concourse.bass_utils import run_bass_kernel_spmd

    nc = _get_nc()
    in_maps = make_in_maps(**inputs)
    res = run_bass_kernel_spmd(nc, in_maps, core_ids=list(range(N_CORES)))
    out = np.concatenate([r["y"].reshape(-1) for r in res.results])
    return out.reshape(B, 1).astype(np.float32)


# revision 13
# speedup vs baseline: 1.8656x; 1.0497x over previous
"""DLRM forward (embedding gather + tiny MLPs) as a Bass/Tile kernel on 8 trn2 cores.

Sharding: data-parallel over the batch (tables replicated in each core's HBM —
total gather traffic is the same as model-parallel but needs no collectives).
Each core computes 2048 rows end-to-end and returns [1, 2048] sigmoid outputs.

v6 (v5 44.3µs, v4 46.9µs, v3 62.5µs, v0 110µs):
  - fp8e4m3 tables (x16) bit-packed as u16 pairs, bf16-typed PE pair
    transposes, DoubleRow fp8 matmuls (256 k per pass).
  - comb-index DMA issued from gpsimd itself so the gather dispatches
    wait on a same-engine semaphore (v5 lost ~2.5µs to a cross-engine
    event-semaphore stuck behind other DMAs on the sync queue).
  - warm-up matmuls during the first-gather wait: the PE clock governor
    (HAM) ignores transpose-mode work, and v5 ran its first ~15µs at
    low clock before the boost engaged.
  - per group: one dense transpose+matmul phase (fewer PE mode switches),
    bottom-MLP for group g+1 computed during group g, relu/copy staging
    split DVE/ACT by measured occupancy.
"""

import numpy as np
import ml_dtypes

import concourse.bass as bass
import concourse.mybir as mybir
import concourse.tile as tile
from concourse import bacc

P = 128

N_CORES = 8
B = 16384
F = 26
D = 64
DENSE = 13
DENSE_A = DENSE + 1           # host-appended ones column
CARD = 100000
H_BOT = 8
H_TOP = 16

f32 = mybir.dt.float32
i32 = mybir.dt.int32
fp16 = mybir.dt.float16
fp8 = mybir.dt.float8e4
u16 = mybir.dt.uint16
bf16 = mybir.dt.bfloat16

B_LOC = B // N_CORES          # 2048
K_EMB = F * D                 # 1664
GROUP = 512
TPG = GROUP // P              # 4
N_G = B_LOC // GROUP          # 4
N_T = B_LOC // P              # 16

W_T = F * D // 2              # 832 u16 per sample-tile row
CW = [128] * 6 + [64]         # u16 chunk widths (pairs of fp8 k-values)

FP8_SCALE = 16.0              # tables and tw1 each scaled by this
N_WARM = 16                   # PE warm-up matmuls during the gather wait


def build_kernel():
    nc = bacc.Bacc("TRN2", target_bir_lowering=False)
    comb_d = nc.dram_tensor("sparse_x", [P, N_T * F], i32, kind="ExternalInput")
    dense_d = nc.dram_tensor("dense_x", [P, N_T * DENSE_A], fp16, kind="ExternalInput")
    tables_d = nc.dram_tensor("tables", [F * CARD, D // 2], u16, kind="ExternalInput")
    ident_d = nc.dram_tensor("ident", [P, P], fp16, kind="ExternalInput")
    identu_d = nc.dram_tensor("identu", [P, P], bf16, kind="ExternalInput")
    w1a_d = nc.dram_tensor("w1a", [DENSE_A, H_BOT], fp16, kind="ExternalInput")
    w2a_d = nc.dram_tensor("w2a", [H_BOT + 1, D], fp16, kind="ExternalInput")
    tw1dr_d = nc.dram_tensor("tw1dr", [P, 7 * 2 * H_TOP], fp8, kind="ExternalInput")
    tw1da_d = nc.dram_tensor("tw1da", [D + 1, H_TOP], fp16, kind="ExternalInput")
    tw2_d = nc.dram_tensor("tw2", [H_TOP, 1], fp16, kind="ExternalInput")
    tb2_d = nc.dram_tensor("tb2", [1], f32, kind="ExternalInput")
    y_d = nc.dram_tensor("y", [1, B_LOC], f32, kind="ExternalOutput")

    with tile.TileContext(nc) as tc:
        with (
            tc.tile_pool(name="const", bufs=1) as cpool,
            tc.tile_pool(name="embt", bufs=3) as embtp,
            tc.tile_pool(name="small", bufs=3) as smallp,
            tc.tile_pool(name="ptr", bufs=3, space="PSUM") as ptrp,
            tc.tile_pool(name="po1", bufs=2, space="PSUM") as po1p,
            tc.tile_pool(name="pwarm", bufs=1, space="PSUM") as pwarmp,
            tc.tile_pool(name="psmall", bufs=2, space="PSUM") as psmallp,
        ):
            # ---- index DMA from gpsimd: gathers wait on a SAME-ENGINE sem ----
            comb_sb = cpool.tile([P, N_T * F], i32)
            nc.gpsimd.dma_start(out=comb_sb[:], in_=comb_d[:, :])

            # ---- gather dispatches (gpsimd only): group 0 split for latency ----
            embs = [cpool.tile([P, TPG * W_T], u16, name=f"emb{g}") for g in range(N_G)]
            spans = [(0, 0, 2 * F), (0, 2 * F, 4 * F)] + [
                (g, 0, TPG * F) for g in range(1, N_G)
            ]
            for g, f0, f1 in spans:
                nc.gpsimd.indirect_dma_start(
                    out=embs[g][:, f0 * (D // 2) : f1 * (D // 2)],
                    out_offset=None,
                    in_=tables_d[:, :],
                    in_offset=bass.IndirectOffsetOnAxis(
                        ap=comb_sb[:, g * TPG * F + f0 : g * TPG * F + f1], axis=0
                    ),
                )

            # ---- weights (host-prepped, contiguous), spread over both HWDGE
            # queues in need-order ----
            dense_sb = cpool.tile([P, N_T * DENSE_A], fp16)
            nc.sync.dma_start(out=dense_sb[:], in_=dense_d[:, :])
            ident = cpool.tile([P, P], fp16)
            nc.sync.dma_start(out=ident[:], in_=ident_d[:, :])
            w1a_sb = cpool.tile([DENSE_A, H_BOT], fp16)
            nc.scalar.dma_start(out=w1a_sb[:], in_=w1a_d[:, :])
            identu = cpool.tile([P, P], bf16)
            nc.scalar.dma_start(out=identu[:], in_=identu_d[:, :])
            w2a_sb = cpool.tile([H_BOT + 1, D], fp16)
            nc.scalar.dma_start(out=w2a_sb[:], in_=w2a_d[:, :])
            tw1dr_sb = cpool.tile([P, 7 * 2 * H_TOP], fp8)
            nc.scalar.dma_start(out=tw1dr_sb[:], in_=tw1dr_d[:, :])
            tw1da_sb = cpool.tile([D + 1, H_TOP], fp16)
            nc.scalar.dma_start(out=tw1da_sb[:], in_=tw1da_d[:, :])
            tw2_sb = cpool.tile([H_TOP, 1], fp16)
            nc.scalar.dma_start(out=tw2_sb[:], in_=tw2_d[:, :])
            tb2_sb = cpool.tile([1, 1], f32)
            nc.scalar.dma_start(out=tb2_sb[:], in_=tb2_d[:, None])

            y_row = cpool.tile([1, B_LOC], f32)
            dta = [cpool.tile([D + 1, GROUP], fp16, name=f"dta{g}") for g in range(N_G)]
            ha2 = [cpool.tile([H_BOT + 1, GROUP], fp16, name=f"ha{i}") for i in range(2)]
            for t in ha2 + dta[:2]:
                nc.vector.memset(t[:], 1.0)
            for t in dta[2:]:
                nc.gpsimd.memset(t[:], 1.0)

            # ---- dense bottom-MLP pieces (biases folded via ones-rows;
            # dxt's ones row comes in with the host data) ----
            def dense_tr(g):
                pdx_t = psmallp.tile([DENSE_A, GROUP], fp16, tag="psmall")
                for j in range(TPG):
                    t = g * TPG + j
                    nc.tensor.transpose(
                        out=pdx_t[:, bass.ts(j, P)],
                        in_=dense_sb[:, bass.ts(t, DENSE_A)],
                        identity=ident[:],
                    )
                return pdx_t

            def dense_mm1(g, pdx_t):
                dxt = smallp.tile([DENSE_A, GROUP], fp16, tag="dxt")
                nc.vector.tensor_copy(out=dxt[:], in_=pdx_t[:])
                ph = psmallp.tile([H_BOT, GROUP], f32, tag="psmall")
                nc.tensor.matmul(out=ph[:], lhsT=w1a_sb[:], rhs=dxt[:], start=True, stop=True)
                return ph

            def dense_mm2(g, ph):
                ha = ha2[g % 2]
                nc.scalar.activation(
                    out=ha[0:H_BOT, :], in_=ph[:],
                    func=mybir.ActivationFunctionType.Relu,
                )
                pd = psmallp.tile([D, GROUP], f32, tag="psmall")
                nc.tensor.matmul(out=pd[:], lhsT=w2a_sb[:], rhs=ha[:], start=True, stop=True)
                nc.scalar.activation(
                    out=dta[g][0:D, :], in_=pd[:],
                    func=mybir.ActivationFunctionType.Copy,
                )

            # group 0+1 dense runs up front, under the first gather; the PE
            # warm-up matmuls engage the HAM clock boost (transposes don't
            # count as PE-busy for it)
            pwarm = pwarmp.tile([H_TOP, GROUP], f32, tag="pwarm")
            pdx0 = dense_tr(0)
            ph0 = dense_mm1(0, pdx0)
            pdx1 = dense_tr(1)
            dense_mm2(0, ph0)
            ph1 = dense_mm1(1, pdx1)
            dense_mm2(1, ph1)
            for _ in range(N_WARM):
                nc.tensor.matmul(
                    out=pwarm[:, 0:224], lhsT=ident[:, 0:H_TOP], rhs=dense_sb[:, 0:224],
                    start=True, stop=True,
                )

            # ---- fp8-pair transposes + DoubleRow top-MLP accumulation ----
            tail = []  # deferred (o1 relu, tw2 matmul, sigmoid) of prev group

            def flush_tail():
                while tail:
                    tail.pop(0)()

            for g in range(N_G):
                po1 = po1p.tile([H_TOP, GROUP], f32, tag="po1")
                mms = []
                casted = []

                def emit_tchunks(dc, g=g, mms=mms, casted=casted, po1=po1):
                    # transposes for two k-chunks into one PSUM tile
                    cs = [2 * dc] + ([2 * dc + 1] if 2 * dc + 1 < 7 else [])
                    ptr_t = ptrp.tile([P, 2 * GROUP], bf16, tag="ptr")
                    for ci, c in enumerate(cs):
                        cw = CW[c]
                        for j in range(TPG):
                            o = j * W_T + c * 128
                            nc.tensor.transpose(
                                out=ptr_t[0:cw, ci * GROUP + j * P : ci * GROUP + (j + 1) * P],
                                in_=embs[g][:, o : o + cw].bitcast(bf16),
                                identity=identu[:],
                            )
                    casted.append((dc, cs, ptr_t))

                def emit_cast(g=g, mms=mms, casted=casted, po1=po1):
                    dc, cs, ptr_t = casted.pop(0)
                    embt = embtp.tile([P, 2 * GROUP], bf16, tag="embt")
                    wid = len(cs) * GROUP
                    if dc % 2 == 0:
                        nc.vector.tensor_copy(out=embt[:, 0:wid], in_=ptr_t[:, 0:wid])
                    else:
                        nc.scalar.activation(
                            out=embt[:, 0:wid], in_=ptr_t[:, 0:wid],
                            func=mybir.ActivationFunctionType.Copy,
                        )
                    for ci, c in enumerate(cs):
                        def mm(c=c, ci=ci, embt=embt):
                            cw = CW[c]
                            lhsT = tw1dr_sb[0:cw, c * 32 : (c + 1) * 32].rearrange(
                                "p (i m) -> p i m", i=2
                            )
                            rhs = embt[0:cw, bass.ts(ci, GROUP)].bitcast(fp8).rearrange(
                                "p (n i) -> p i n", i=2
                            )
                            nc.tensor.matmul(
                                out=po1[:], lhsT=lhsT, rhs=rhs,
                                start=(c == 0), stop=False,
                                perf_mode=mybir.MatmulPerfMode.DoubleRow,
                            )
                        mms.append(mm)

                # transpose phase (casts trail by one dchunk), then matmul phase
                emit_tchunks(0)
                emit_cast()
                emit_tchunks(1)
                if g < N_G - 2:
                    pdx_t = dense_tr(g + 2)
                emit_cast()
                flush_tail()
                emit_tchunks(2)
                emit_cast()
                mms.pop(0)()                      # MM(0)
                mms.pop(0)()                      # MM(1)
                if g < N_G - 2:
                    ph = dense_mm1(g + 2, pdx_t)
                emit_tchunks(3)                   # single chunk 6
                emit_cast()
                mms.pop(0)()                      # MM(2)
                mms.pop(0)()                      # MM(3)
                if g < N_G - 2:
                    dense_mm2(g + 2, ph)
                mms.pop(0)()                      # MM(4)
                mms.pop(0)()                      # MM(5)
                mms.pop(0)()                      # MM(6)
                nc.tensor.matmul(
                    out=po1[:], lhsT=tw1da_sb[:], rhs=dta[g][:], start=False, stop=True
                )

                def make_tail(g=g, po1=po1):
                    halves = 2 if g == N_G - 1 else 1
                    def run():
                        o1 = smallp.tile([H_TOP, GROUP], fp16, tag="o1")
                        w = GROUP // halves
                        plgs = []
                        for h in range(halves):
                            nc.vector.tensor_scalar(
                                out=o1[:, h * w : (h + 1) * w],
                                in0=po1[:, h * w : (h + 1) * w],
                                scalar1=1.0 / (FP8_SCALE * FP8_SCALE), scalar2=0.0,
                                op0=mybir.AluOpType.mult, op1=mybir.AluOpType.max,
                            )
                            plg = psmallp.tile([1, GROUP], f32, tag="psmall")
                            nc.tensor.matmul(
                                out=plg[:, 0:w], lhsT=tw2_sb[:],
                                rhs=o1[:, h * w : (h + 1) * w], start=True, stop=True,
                            )
                            plgs.append(plg)
                        for h, plg in enumerate(plgs):
                            nc.scalar.activation(
                                out=y_row[:, g * GROUP + h * w : g * GROUP + (h + 1) * w],
                                in_=plg[:, 0:w],
                                func=mybir.ActivationFunctionType.Sigmoid,
                                bias=tb2_sb[:],
                            )
                        nc.sync.dma_start(
                            out=y_d[:, bass.ts(g, GROUP)],
                            in_=y_row[:, bass.ts(g, GROUP)],
                        )
                    return run

                tail.append(make_tail())
            flush_tail()

    nc.compile()
    return nc


_NC_CACHE = {}


def _get_nc():
    if "nc" not in _NC_CACHE:
        _NC_CACHE["nc"] = build_kernel()
    return _NC_CACHE["nc"]


FP8_NP = ml_dtypes.float8_e4m3  # what mybir.dt.float8e4 maps to


def make_in_maps(dense_x, sparse_x, tables, w1, b1, w2, b2, tw1, tb1, tw2, tb2):
    s = FP8_SCALE
    t8 = (np.asarray(tables, np.float32).reshape(F * CARD, D) * s).astype(FP8_NP)
    tables_u16 = np.ascontiguousarray(t8).view(np.uint16)  # [V, 32]
    comb = np.asarray(sparse_x).astype(np.int32) + (
        np.arange(F, dtype=np.int32) * CARD
    )[None, :]
    dense_f = np.asarray(dense_x).astype(np.float16)
    dense_aug = np.concatenate(
        [dense_f, np.ones((dense_f.shape[0], 1), np.float16)], axis=1
    )  # [B, 14] with ones column
    tw1 = np.asarray(tw1, np.float32)
    # tw1dr[p, c*32 + i*16 + m] = fp8(s * tw1[c*256 + 2p + i, m])
    tw1dr = np.zeros((P, 7 * 2 * H_TOP), dtype=FP8_NP)
    for c in range(7):
        rows = 2 * CW[c]
        blk = (tw1[c * 256 : c * 256 + rows] * s).astype(FP8_NP)  # [rows, 16]
        blk = blk.reshape(CW[c], 2, H_TOP).reshape(CW[c], 2 * H_TOP)
        tw1dr[0 : CW[c], c * 32 : (c + 1) * 32] = blk
    tw1da = np.vstack(
        [tw1[K_EMB : K_EMB + D], np.asarray(tb1, np.float32)[None, :]]
    ) * (s * s)
    shared = {
        "tables": tables_u16,
        "ident": np.eye(P, dtype=np.float16),
        "identu": np.eye(P, dtype=ml_dtypes.bfloat16),
        "w1a": np.vstack([np.asarray(w1, np.float32), np.asarray(b1, np.float32)[None, :]]).astype(np.float16),
        "w2a": np.vstack([np.asarray(w2, np.float32), np.asarray(b2, np.float32)[None, :]]).astype(np.float16),
        "tw1dr": tw1dr,
        "tw1da": tw1da.astype(np.float16),
        "tw2": np.asarray(tw2, np.float32).astype(np.float16),
        "tb2": np.ascontiguousarray(np.asarray(tb2, np.float32)),
    }
    in_maps = []
    for c in range(N_CORES):
        m = dict(shared)
        # host pre-transpose: [p, (t f)] so the device DMA is contiguous
        dl = dense_aug[c * B_LOC : (c + 1) * B_LOC]
        m["dense_x"] = np.ascontiguousarray(
            dl.reshape(N_T, P, DENSE_A).transpose(1, 0, 2).reshape(P, N_T * DENSE_A)
        )
        cl = comb[c * B_LOC : (c + 1) * B_LOC]
        m["sparse_x"] = np.ascontiguousarray(
            cl.reshape(N_T, P, F).transpose(1, 0, 2).reshape(P, N_T * F)
        )
        in_maps.append(m)
    return in_maps


def kernel(**inputs):
    from concourse.bass_utils import run_bass_kernel_spmd

    nc = _get_nc()
    in_maps = make_in_maps(**inputs)
    res = run_bass_kernel_spmd(nc, in_maps, core_ids=list(range(N_CORES)))
    out = np.concatenate([r["y"].reshape(-1) for r in res.results])
    return out.reshape(B, 1).astype(np.float32)
